# revision 25
# baseline (speedup 1.0000x reference)
"""GQA attention (B=4, L=1024, D=4096, 32 Q heads / 8 KV heads, head_dim=128,
traditional RoPE, causal mask) on 8 TRN2 NeuronCores.

Sharding: tensor-parallel over heads. Core c owns Q heads {c, c+8, c+16, c+24}
(all map to KV head c under the reference's jnp.tile GQA expansion) — so each
core needs exactly one KV head. wq/wk/wv are column-sharded, wo row-sharded,
x replicated. Each core computes a partial output (its heads' contribution
through wo); the host sums the 8 partials.

On-chip layout: everything transposed. The projection computes q^T/k^T/v^T
([head_dim, tokens], head_dim on partitions) directly, which is exactly the
lhsT/rhs layout the scores matmul (s^T = k^T.T-contract) and the output
projection (lhsT = attn^T) want, so no activation-sized transposes are needed.
RoPE in transposed layout mixes partition pairs; that's done with one
128x128 pair-swap permutation matmul plus two elementwise muls against host
cos/sin tables. Softmax runs without max-subtraction (scores ~ N(0, 1.3^2)),
sums via a ones-vector matmul, normalization by an outer-product broadcast
of 1/sum. Causal structure: fully-masked 128x512 score blocks are skipped,
diagonal blocks are zeroed after exp with a binary mask (host-verified that
the mask is a pure 0/-1e9 mask).
"""

import numpy as np
import ml_dtypes
from contextlib import ExitStack

import concourse.bass as bass
import concourse.mybir as mybir
import concourse.tile as tile
from concourse import bacc
from concourse.bass_utils import run_bass_kernel_spmd

DIM = 4096
N_HEADS = 32
N_KV = 8
DH = 128
B, L = 4, 1024
NCORES = 8
HPC = N_HEADS // NCORES  # 4 q-heads per core
T = B * L  # 4096 tokens total
SCALE = DH ** -0.5
ROPE_BASE = 10000.0

BF = mybir.dt.bfloat16
F32 = mybir.dt.float32
NPBF = ml_dtypes.bfloat16

# number of 512-token q chunks per batch, 128-token k tiles per batch
QC = L // 512  # 2
KT = L // 128  # 8

TRACE = False
LAST_RESULT = [None]


def _classify_blocks(mask):
    """Per (kt, qc) block of mask^T: 'skip' (all masked), 'free' (no mask),
    or 'mixed'. Host-side; the kernel structure is specialized to this."""
    maskT = np.asarray(mask).T
    assert np.all((maskT == 0.0) | (maskT <= -1e8)), (
        "kernel assumes a binary additive mask (0 / -1e9)"
    )
    cls = {}
    for qc in range(QC):
        for kt in range(KT):
            blk = maskT[kt * 128:(kt + 1) * 128, qc * 512:(qc + 1) * 512]
            if np.all(blk <= -1e8):
                cls[(kt, qc)] = "skip"
            elif np.all(blk == 0.0):
                cls[(kt, qc)] = "free"
            else:
                cls[(kt, qc)] = "mixed"
    return cls


def _build(cls):
    nc = bacc.Bacc(
        "TRN2", target_bir_lowering=False, debug=False, num_devices=NCORES
    )

    # weights come in host-pre-tiled partition-major layout [128, ...] so
    # each loads as 128 large contiguous DMA descriptors
    NDT_ = DIM // 128
    xT = nc.dram_tensor("xT", [DIM, T], BF, kind="ExternalInput").ap()
    wq = nc.dram_tensor("wq", [128, NDT_ * HPC * DH], BF, kind="ExternalInput").ap()
    wk = nc.dram_tensor("wk", [128, NDT_ * DH], BF, kind="ExternalInput").ap()
    wv = nc.dram_tensor("wv", [128, NDT_ * DH], BF, kind="ExternalInput").ap()
    wo = nc.dram_tensor("wo", [128, HPC * DIM], BF, kind="ExternalInput").ap()
    mbinT = nc.dram_tensor("mbinT", [L, L], BF, kind="ExternalInput").ap()
    cos2 = nc.dram_tensor("cos2", [DH, L], BF, kind="ExternalInput").ap()
    sin2 = nc.dram_tensor("sin2", [DH, L], BF, kind="ExternalInput").ap()
    pswap = nc.dram_tensor("pswap", [DH, DH], BF, kind="ExternalInput").ap()
    ident = nc.dram_tensor("ident", [DH, DH], BF, kind="ExternalInput").ap()
    out = nc.dram_tensor("out", [T, DIM], F32, kind="ExternalOutput").ap()

    xT_r = xT.rearrange("(dt p) t -> dt p t", p=128)  # [32, 128, 4096]
    NDT = DIM // 128  # 32 contraction tiles

    with TileCtx(nc) as tc, ExitStack() as ctx:
        persist = ctx.enter_context(tc.tile_pool(name="persist", bufs=1))
        qt_pool = ctx.enter_context(tc.tile_pool(name="qt", bufs=HPC * B))
        kt_pool = ctx.enter_context(tc.tile_pool(name="kt", bufs=B))
        v_pool = ctx.enter_context(tc.tile_pool(name="v", bufs=B))

        cos_sb = persist.tile([DH, L], BF)
        sin_sb = persist.tile([DH, L], BF)
        psw_sb = persist.tile([DH, DH], BF)
        idn_sb = persist.tile([DH, DH], BF)
        ones_sb = persist.tile([128, 128], BF)
        nc.vector.memset(ones_sb, 1.0)

        def _load_tables():
            nc.sync.dma_start(out=cos_sb, in_=cos2)
            nc.sync.dma_start(out=sin_sb, in_=sin2)
            nc.sync.dma_start(out=psw_sb, in_=pswap)
            nc.sync.dma_start(out=idn_sb, in_=ident)

        # wo + mask tiles live in outer pools (created before stage A's pools)
        # so their SBUF addresses don't overlap stage-A tiles; their DMAs are
        # emitted at the end of stage A so they don't delay the A-critical
        # weight/x loads at kernel start.
        wo_p = ctx.enter_context(tc.tile_pool(name="wo_p", bufs=1))
        mp = ctx.enter_context(tc.tile_pool(name="mp", bufs=8))
        wo_sb = wo_p.tile([128, HPC, DIM], BF)
        msk_sb = {}
        for (kt, qc), c in cls.items():
            if c == "mixed":
                msk_sb[(kt, qc)] = mp.tile([128, 512], BF, name="mtile")

        qt_t = [[None] * B for _ in range(HPC)]  # [128 dh, 1024 t] per (h, b)
        kt_t = [None] * B                        # [128 dh, 1024 t]
        v_t = [None] * B                         # [128 t, 8, 128 dh]

        # ---------------- Stage A: QKV projection + RoPE ----------------
        with tc.tile_pool(name="wA", bufs=1) as wA, \
             tc.tile_pool(name="xp", bufs=8) as xp, \
             tc.tile_pool(name="evac", bufs=8) as evac, \
             tc.tile_pool(name="rtmp", bufs=8) as rtmp, \
             tc.tile_pool(name="psA", bufs=6, space="PSUM") as psA, \
             tc.tile_pool(name="psS", bufs=1, space="PSUM") as psS:

            # weights drip-feed in 8 chunks of 4 d-tiles so the first matmuls
            # start after ~0.5 MiB of DMA and never outrun the weight stream
            WCH = 4
            NCH = NDT // WCH
            wq_r = wq.rearrange("p (ch dt m) -> p ch dt m", ch=NCH, dt=WCH)
            wk_r = wk.rearrange("p (ch dt m) -> p ch dt m", ch=NCH, dt=WCH)
            wv_r = wv.rearrange("p (ch dt m) -> p ch dt m", ch=NCH, dt=WCH)
            wq_c, wk_c, wv_c = [], [], []
            for i in range(NCH):
                wk_c.append(wA.tile([128, WCH, DH], BF, name=f"wk{i}"))
                wv_c.append(wA.tile([128, WCH, DH], BF, name=f"wv{i}"))
                wq_c.append(wA.tile([128, WCH, HPC * DH], BF, name=f"wq{i}"))

            def _load_wchunk(i):
                nc.sync.dma_start(out=wk_c[i], in_=wk_r[:, i])
                nc.sync.dma_start(out=wv_c[i], in_=wv_r[:, i])
                nc.sync.dma_start(out=wq_c[i], in_=wq_r[:, i])

            _load_wchunk(0)

            def wsl(ts, d):
                return ts[d // WCH][:, d % WCH]

            for tci in range(T // 512):  # 8 chunks of 512 tokens
                b, half = tci // 2, tci % 2
                lsl = slice(half * 512, (half + 1) * 512)  # pos within batch
                if half == 0:
                    for h in range(HPC):
                        qt_t[h][b] = qt_pool.tile([DH, L], BF, name="qtile")
                    kt_t[b] = kt_pool.tile([DH, L], BF, name="ktile")
                    v_t[b] = v_pool.tile([128, KT, DH], BF, name="vtile")

                ps_q = [psA.tile([128, 512], F32, name="psacc") for _ in range(HPC)]
                ps_k = psA.tile([128, 512], F32, name="psacc")
                ps_v = psA.tile([128, 512], F32, name="psacc")
                for d in range(NDT):
                    # drip-feed weight chunks between x-tile loads on the
                    # first pass so neither stream starves the other
                    if tci == 0 and d % WCH == 0 and d // WCH + 1 < NCH:
                        _load_wchunk(d // WCH + 1)
                    if tci == 0 and d == 4:
                        _load_tables()
                    xt = xp.tile([128, 512], BF)
                    nc.sync.dma_start(
                        out=xt, in_=xT_r[d, :, tci * 512:(tci + 1) * 512]
                    )
                    st, sp = d == 0, d == NDT - 1
                    wqd = wsl(wq_c, d)
                    for h in range(HPC):
                        nc.tensor.matmul(
                            ps_q[h], wqd[:, h * DH:(h + 1) * DH], xt,
                            start=st, stop=sp,
                        )
                    nc.tensor.matmul(ps_k, wsl(wk_c, d), xt, start=st, stop=sp)
                    nc.tensor.matmul(ps_v, wsl(wv_c, d), xt, start=st, stop=sp)

                # RoPE on q heads and k: r = raw*cos + (P raw)*sin
                for h in range(HPC + 1):
                    ps = ps_k if h == HPC else ps_q[h]
                    dst = kt_t[b] if h == HPC else qt_t[h][b]
                    raw = evac.tile([128, 512], BF, name="raw")
                    nc.scalar.copy(raw, ps)
                    ps_sw = psS.tile([128, 512], F32, name="pssw")
                    nc.tensor.matmul(ps_sw, psw_sb, raw, start=True, stop=True)
                    t1 = rtmp.tile([128, 512], BF, name="t1")
                    t2 = rtmp.tile([128, 512], BF, name="t2")
                    nc.vector.tensor_mul(t1, raw, cos_sb[:, lsl])
                    nc.vector.tensor_mul(t2, ps_sw, sin_sb[:, lsl])
                    nc.vector.tensor_add(dst[:, lsl], t1, t2)

                # v: transpose [dh, t] -> [t, dh] natural, 128 cols at a time
                vraw = evac.tile([128, 512], BF, name="raw")
                nc.scalar.copy(vraw, ps_v)
                for s in range(4):
                    ps_t = psS.tile([128, 128], BF, name="pstr")
                    nc.tensor.transpose(ps_t, vraw[:, s * 128:(s + 1) * 128], idn_sb)
                    nc.vector.tensor_copy(v_t[b][:, half * 4 + s], ps_t)

                if tci == 0:
                    # B/C-stage constants: emitted here (not at kernel start)
                    # so they queue behind the A-critical first loads.
                    nc.sync.dma_start(
                        out=wo_sb, in_=wo.rearrange("p (h n) -> p h n", h=HPC)
                    )
                    for (kt, qc), m in msk_sb.items():
                        nc.sync.dma_start(
                            out=m,
                            in_=mbinT[
                                kt * 128:(kt + 1) * 128, qc * 512:(qc + 1) * 512
                            ],
                        )

        # ---------------- Stage B + C: attention + output proj ----------------
        with tc.tile_pool(name="ep", bufs=8) as ep, \
             tc.tile_pool(name="attn", bufs=2 * HPC) as attn_p, \
             tc.tile_pool(name="rcp", bufs=2) as rcp, \
             tc.tile_pool(name="oev", bufs=4) as oev, \
             tc.tile_pool(name="psMM", bufs=4, space="PSUM") as psMM, \
             tc.tile_pool(name="psPV", bufs=2, space="PSUM") as psPV, \
             tc.tile_pool(name="psSum", bufs=2, space="PSUM") as psSum:

            attn_t = [[None] * B for _ in range(HPC)]
            for b in range(B):
                for h in range(HPC):
                    at = attn_p.tile([DH, L], BF, name="atile")
                    attn_t[h][b] = at
                    for qc in range(QC):
                        kts = [k for k in range(KT) if cls[(k, qc)] != "skip"]
                        ps_pv = psPV.tile([128, 512], F32, name="pspv")
                        # ones[128,128] lhsT -> every partition gets the k-sum
                        # row, so normalization is a plain elementwise mul
                        # (no 1-partition reciprocal, no broadcast matmul).
                        ps_sum = psSum.tile([128, 512], F32, name="pssum")
                        for gi in range(0, len(kts), 4):
                            grp = kts[gi:gi + 4]
                            e_ts, ps_ss = {}, {}
                            for kt in grp:
                                ps_s = psMM.tile([128, 512], F32, name="mmps")
                                nc.tensor.matmul(
                                    ps_s,
                                    kt_t[b][:, kt * 128:(kt + 1) * 128],
                                    qt_t[h][b][:, qc * 512:(qc + 1) * 512],
                                    start=True, stop=True,
                                )
                                ps_ss[kt] = ps_s
                            for kt in grp:
                                e_t = ep.tile([128, 512], BF, name="etile")
                                nc.scalar.activation(
                                    e_t, ps_ss[kt],
                                    mybir.ActivationFunctionType.Exp,
                                    scale=SCALE,
                                )
                                if cls[(kt, qc)] == "mixed":
                                    nc.vector.tensor_mul(e_t, e_t, msk_sb[(kt, qc)])
                                e_ts[kt] = e_t
                            for kt in grp:
                                st = kt == kts[0]
                                sp = kt == kts[-1]
                                nc.tensor.matmul(
                                    ps_pv, v_t[b][:, kt], e_ts[kt],
                                    start=st, stop=sp,
                                )
                                nc.tensor.matmul(
                                    ps_sum, ones_sb, e_ts[kt],
                                    start=st, stop=sp,
                                )
                        recip = rcp.tile([128, 512], F32, name="recip")
                        nc.vector.reciprocal_approx_fast(recip, ps_sum)
                        nc.vector.tensor_mul(
                            at[:, qc * 512:(qc + 1) * 512], ps_pv, recip
                        )

                # Stage C for batch b: out[t, :] += sum_h attn_h @ wo_h
                for tt in range(KT):  # 8 token tiles of 128
                    for nck in range(DIM // 512):  # 8 n chunks
                        ps_o = psMM.tile([128, 512], F32, name="mmps")
                        for h in range(HPC):
                            nc.tensor.matmul(
                                ps_o,
                                attn_t[h][b][:, tt * 128:(tt + 1) * 128],
                                wo_sb[:, h, nck * 512:(nck + 1) * 512],
                                start=(h == 0), stop=(h == HPC - 1),
                            )
                        o_sb = oev.tile([128, 512], F32, name="osb")
                        if (tt + nck) % 2 == 0:
                            nc.vector.tensor_copy(o_sb, ps_o)
                        else:
                            nc.scalar.copy(o_sb, ps_o)
                        nc.sync.dma_start(
                            out=out[
                                b * L + tt * 128: b * L + (tt + 1) * 128,
                                nck * 512:(nck + 1) * 512,
                            ],
                            in_=o_sb,
                        )
    nc.finalize()
    return nc


def TileCtx(nc):
    return tile.TileContext(nc)


def _host_tables():
    inv = ROPE_BASE ** (-np.arange(0, DH, 2, dtype=np.float64) / DH)  # [64]
    pos = np.arange(L, dtype=np.float64)
    ang = inv[:, None] * pos[None, :]  # [64, L]
    cos2 = np.repeat(np.cos(ang), 2, axis=0)  # [128, L]
    sin = np.sin(ang)
    sin2 = np.empty((DH, L), dtype=np.float64)
    sin2[0::2] = -sin
    sin2[1::2] = sin
    psw = np.zeros((DH, DH), dtype=np.float32)
    idx = np.arange(0, DH, 2)
    psw[idx, idx + 1] = 1.0
    psw[idx + 1, idx] = 1.0
    return (
        cos2.astype(NPBF),
        sin2.astype(NPBF),
        psw.astype(NPBF),
        np.eye(DH, dtype=np.float32).astype(NPBF),
    )


def kernel(x, mask, wq, wk, wv, wo):
    x = np.asarray(x, dtype=np.float32)
    mask = np.asarray(mask, dtype=np.float32)
    wq = np.asarray(wq, dtype=np.float32)
    wk = np.asarray(wk, dtype=np.float32)
    wv = np.asarray(wv, dtype=np.float32)
    wo = np.asarray(wo, dtype=np.float32)

    cls = _classify_blocks(mask)
    nc = _build(cls)

    xT = np.ascontiguousarray(x.reshape(T, DIM).T).astype(NPBF)
    mbinT = np.ascontiguousarray((mask == 0.0).T.astype(NPBF))
    cos2, sin2, psw, idn = _host_tables()

    def _ptile(w):
        # [DIM_or_512, M] -> partition-major [128, (outer M)] host pre-tiling
        k, m = w.shape
        return np.ascontiguousarray(
            w.reshape(k // 128, 128, m).transpose(1, 0, 2).reshape(128, -1)
        ).astype(NPBF)

    in_maps = []
    for c in range(NCORES):
        cols = np.concatenate(
            [np.arange(h * DH, (h + 1) * DH) for h in range(c, N_HEADS, N_KV)]
        )
        in_maps.append({
            "xT": xT,
            "wq": _ptile(wq[:, cols]),
            "wk": _ptile(wk[:, c * DH:(c + 1) * DH]),
            "wv": _ptile(wv[:, c * DH:(c + 1) * DH]),
            "wo": _ptile(wo[cols, :]),
            "mbinT": mbinT,
            "cos2": cos2,
            "sin2": sin2,
            "pswap": psw,
            "ident": idn,
        })

    res = run_bass_kernel_spmd(
        nc, in_maps, core_ids=list(range(NCORES)), trace=TRACE
    )
    LAST_RESULT[0] = res
    outs = res.results
    total = np.zeros((T, DIM), dtype=np.float32)
    for c in range(NCORES):
        total += np.asarray(outs[c]["out"], dtype=np.float32)
    return total.reshape(B, L, DIM)
